# revision 15
# baseline (speedup 1.0000x reference)
"""BiMPM (bilateral multi-perspective matching) kernel for Trainium2 — v2.

Contract: kernel(**inputs) takes the FULL unsharded inputs (as produced by
setup_inputs) and returns the full [B, 2L, 102] output. Internally shards
data-parallel over batch B=8 across 8 NeuronCores; per-core aux tensors
(transposed layouts, norm tables, folded mm weights) are built host-side.

Self-contained: hardcodes B=8, L=128, H=768, P=16.
"""
import sys

sys.path.insert(0, "/opt/trn_rl_repo")

import numpy as np
import ml_dtypes
from contextlib import ExitStack

from concourse import bacc, mybir, masks
import concourse.tile as tile
from concourse.bass_utils import run_bass_kernel_spmd
from concourse.bass import MemorySpace
from concourse import bass

B, L, H, PP, NCH, NF = 8, 128, 768, 16, 6, 102
EPS = 1e-8
F32 = mybir.dt.float32
BF16 = mybir.dt.bfloat16
AX = mybir.AxisListType
OP = mybir.AluOpType
AF = mybir.ActivationFunctionType

# rw table column blocks: [1|ff16 | 1|fb16 | 1|att16 | 1|matt16 | mp16]
BLK_ATT = slice(34, 51)
BLK_MATT = slice(51, 68)
MP0 = 68
# w2tb34 column blocks: [1|att16 | 1|matt16]
W_ATT = slice(0, 17)
W_MATT = slice(17, 34)

# att_max route split: of every 16 iterations, this many go to the DVE
# scalar_tensor_tensor route (DMA row-broadcast fed); the rest go to the
# ACT-multiply (PE broadcast fed) + max route.
DVE_CNT = 5
# max executor for the ACT route: 'gp' or 'dve'
ACT_MAX_ENGINE = "dve"


def _trace_kernel(tc, dins, dout):
    nc = tc.nc
    with ExitStack() as ctx:
        sb = ctx.enter_context(tc.tile_pool(name="sb", bufs=1))
        sc = ctx.enter_context(tc.tile_pool(name="sc", bufs=3))
        ring = ctx.enter_context(tc.tile_pool(name="ring", bufs=1))
        tmpp = ctx.enter_context(tc.tile_pool(name="tmpp", bufs=6))
        ps_t = ctx.enter_context(
            tc.tile_pool(name="ps_t", bufs=2, space=MemorySpace.PSUM))
        ps_bc = ctx.enter_context(
            tc.tile_pool(name="ps_bc", bufs=2, space=MemorySpace.PSUM))
        ps_mm = ctx.enter_context(
            tc.tile_pool(name="ps_mm", bufs=2, space=MemorySpace.PSUM))

        # ---- load inputs ----
        def load(name, shape, dt=F32):
            t = sb.tile(shape, dt, tag=name, name=name + "_t")
            nc.sync.dma_start(t[:], dins[name][:])
            return t

        def load_gp(name, shape, dt=F32):
            t = sb.tile(shape, dt, tag=name, name=name + "_t")
            nc.gpsimd.dma_start(t[:], dins[name][:])
            return t

        c1Tb = load("c1Tb", [L, NCH, L], BF16)
        c2Tb = load("c2Tb", [L, NCH, L], BF16)
        c1b = load("c1b", [L, H], BF16)
        c2b = load("c2b", [L, H], BF16)
        c1xb = load("c1xb", [L, H], BF16)
        c2xb = load("c2xb", [L, H], BF16)
        rhs1 = load("rhs1", [L, NCH, 34], BF16)
        rhs2 = load("rhs2", [L, NCH, 34], BF16)
        w2tb = load("w2tb", [L, NCH, 34], BF16)
        rw1 = load("rw1", [L, 84])
        rw2 = load("rw2", [L, 84])
        mone1f = load("mone1f", [L, L])
        mone2f = load("mone2f", [L, L])
        invl1 = load("invl1", [L, 1])
        invl2 = load("invl2", [L, 1])
        negk1 = load("negk1", [L, 1])
        negk2 = load("negk2", [L, 1])
        wc1T4 = load("wc1T4", [L, NCH, PP, L], BF16)

        identf = sb.tile([L, L], F32, tag="identf")
        masks.make_identity(nc, identf[:])
        identb = sb.tile([L, L], BF16, tag="identb")
        masks.make_identity(nc, identb[:])
        ones_colb = sb.tile([L, 1], BF16, tag="ones_colb")
        nc.vector.memset(ones_colb[:], 1.0)

        out1 = sb.tile([L, NF], F32, tag="out1")
        out2 = sb.tile([L, NF], F32, tag="out2")

        # ---- cos chain ----
        dotsp = ps_t.tile([L, L], F32, tag="t")
        for c in range(NCH):
            nc.tensor.matmul(dotsp[:], c1Tb[:, c, :], c2Tb[:, c, :],
                             start=(c == 0), stop=(c == NCH - 1))
        wS = sc.tile([L, L], F32, tag="wS")
        nc.vector.tensor_scalar(wS[:], dotsp[:], rw1[:, 0:1], None, OP.mult)
        wTp = ps_t.tile([L, L], F32, tag="t")
        nc.tensor.transpose(wTp[:], wS[:], identf[:])

        cosT = sb.tile([L, L], F32, tag="cosT")
        nc.vector.tensor_scalar(cosT[:], wTp[:], rw2[:, 0:1], None, OP.mult)
        s2sum = sc.tile([L, 1], F32, tag="s2sum")
        cosMTf = sb.tile([L, L], F32, tag="cosMTf")
        nc.vector.scalar_tensor_tensor(cosMTf[:], wTp[:], rw2[:, 0:1],
                                       mone1f[:], OP.mult, OP.add)
        nc.vector.reduce_sum(s2sum[:], cosMTf[:], axis=AX.X)
        # cmax2 = rmax_i(wTp) * rw2 ; cmean2 = (s2sum + negk1) * invl1
        mx2 = sc.tile([L, 1], F32, tag="mx2")
        nc.vector.reduce_max(mx2[:], wTp[:], axis=AX.X)
        nc.vector.tensor_scalar(out2[:, 0:1], mx2[:], rw2[:, 0:1], None,
                                OP.mult)
        nc.vector.tensor_scalar(s2sum[:], s2sum[:], negk1[:, 0:1], None,
                                OP.add)
        nc.vector.tensor_scalar(out2[:, 1:2], s2sum[:], invl1[:, 0:1], None,
                                OP.mult)
        cosMTb = sb.tile([L, L], BF16, tag="cosMTb")
        nc.scalar.copy(cosMTb[:], cosMTf[:])

        cosp = ps_t.tile([L, L], F32, tag="t")
        nc.tensor.transpose(cosp[:], cosT[:], identf[:])
        s1sum = sc.tile([L, 1], F32, tag="s1sum")
        cosMf = sb.tile([L, L], F32, tag="cosMf")
        nc.vector.scalar_tensor_tensor(cosMf[:], cosp[:], 1.0, mone2f[:],
                                       OP.mult, OP.add)
        nc.vector.reduce_sum(s1sum[:], cosMf[:], axis=AX.X)
        nc.vector.reduce_max(out1[:, 0:1], cosp[:], axis=AX.X)
        nc.vector.tensor_scalar(s1sum[:], s1sum[:], negk2[:, 0:1], None,
                                OP.add)
        nc.vector.tensor_scalar(out1[:, 1:2], s1sum[:], invl2[:, 0:1], None,
                                OP.mult)
        cosMb = sb.tile([L, L], BF16, tag="cosMb")
        nc.scalar.copy(cosMb[:], cosMf[:])

        # ---- ff/bf matvec features -> out[:, 2:36] ----
        def ff_feats(cTb, rhs, rw, out):
            ffp = ps_mm.tile([L, 34], F32, tag="mm")
            for c in range(NCH):
                nc.tensor.matmul(ffp[:], cTb[:, c, :], rhs[:, c, :],
                                 start=(c == 0), stop=(c == NCH - 1))
            nc.vector.tensor_tensor(out[:, 2:36], ffp[:], rw[:, 0:34],
                                    op=OP.mult)

        ff_feats(c1Tb, rhs1, rw1, out1)
        ff_feats(c2Tb, rhs2, rw2, out2)

        # ---- attentive mean (softmax over H of cos @ ctx) -> bf16 ----
        def att_mean(lhsT, rhs, tag):
            sp = ps_bc.tile([L, H], F32, tag="bc")
            nc.tensor.matmul(sp[:, 0:512], lhsT[:], rhs[:, 0:512],
                             start=True, stop=True)
            nc.tensor.matmul(sp[:, 512:H], lhsT[:], rhs[:, 512:H],
                             start=True, stop=True)
            am = sb.tile([L, H], BF16, tag=tag, name=tag + "_t")
            se = sc.tile([L, 1], F32, tag="se")
            nc.scalar.activation(am[:], sp[:], AF.Exp,
                                 scale=1.0, accum_out=se[:, 0:1])
            rse = sc.tile([L, 1], F32, tag="rse")
            nc.vector.reciprocal(rse[:], se[:])
            nc.vector.tensor_scalar(am[:], am[:], rse[:, 0:1], None, OP.mult)
            return am

        am2 = att_mean(cosMTb, c2b, "am2")   # [i,H]
        am1 = att_mean(cosMb, c1b, "am1")    # [j,H]

        # ---- am/amx rowwise mpm feature blocks ----
        def mpm_block_units(v, cTb, rw_side, wblk, out, col0):
            vsqTb = sc.tile([L, NCH, L], BF16, tag="vsqTb")
            prTb = sc.tile([L, NCH, L], BF16, tag="prTb")

            def chunk(c):
                tp = ps_t.tile([L, L], BF16, tag="t")
                nc.tensor.transpose(tp[:], v[:, c * L:(c + 1) * L], identb[:])
                nc.scalar.square(vsqTb[:, c, :], tp[:])
                nc.vector.tensor_tensor(prTb[:, c, :], cTb[:, c, :],
                                        tp[:], op=OP.mult)

            nump = ps_mm.tile([L, 17], F32, tag="mm")
            wnp = ps_mm.tile([L, 17], F32, tag="mm")

            def mms(c):
                nc.tensor.matmul(nump[:], prTb[:, c, :], w2tb[:, c, wblk],
                                 start=(c == 0), stop=(c == NCH - 1))
                nc.tensor.matmul(wnp[:], vsqTb[:, c, :], w2tb[:, c, wblk],
                                 start=(c == 0), stop=(c == NCH - 1))

            def fin():
                rwv = sc.tile([L, 17], F32, tag="rwv")
                nc.scalar.sqrt(rwv[:], wnp[:])
                nc.vector.tensor_scalar(rwv[:], rwv[:], EPS, None, OP.max)
                nc.vector.reciprocal(rwv[:], rwv[:])
                ft = sc.tile([L, 17], F32, tag="ft")
                nc.vector.tensor_tensor(ft[:], nump[:], rw_side[:, BLK_ATT if col0 == 68 else BLK_MATT], op=OP.mult)
                nc.vector.tensor_tensor(out[:, col0:col0 + 17], ft[:], rwv[:],
                                        op=OP.mult)

            return [lambda c=c: chunk(c) for c in range(NCH)] + \
                   [lambda c=c: mms(c) for c in range(NCH)] + [fin]

        # ---- mm (pairwise multi-perspective) block units ----
        def mm_units():
            units = []
            num4s = {}
            for g in range(4):
                def grp(g=g):
                    num4 = ps_mm.tile([L, 4, L], F32, tag="mm", name=f"num4_{g}")
                    for c in range(NCH):
                        nc.tensor.matmul(
                            num4[:], c2Tb[:, c, :],
                            wc1T4[:, c, 4 * g:4 * g + 4, :],
                            start=(c == 0), stop=(c == NCH - 1))
                    num4s[g] = num4
                units.append(grp)
                for pi in range(4):
                    def per_p(g=g, pi=pi):
                        p = 4 * g + pi
                        num4 = num4s[g]
                        mhat = sc.tile([L, L], BF16, tag="mhat")
                        m2s = sc.tile([L, 1], F32, tag="m2s")
                        nc.vector.tensor_scalar(
                            mhat[:], num4[:, pi, :],
                            rw2[:, MP0 + p:MP0 + p + 1], None, OP.mult)
                        nc.vector.reduce_sum(m2s[:], mhat[:], axis=AX.X)
                        # side 2 (over i, free dim of mhat[j,i])
                        nc.vector.reduce_max(out2[:, 36 + p:37 + p], mhat[:],
                                             axis=AX.X)
                        nc.vector.tensor_scalar(out2[:, 52 + p:53 + p], m2s[:],
                                                invl1[:, 0:1], None, OP.mult)
                        # side 1: transpose to [i,j]
                        mT = ps_t.tile([L, L], BF16, tag="t")
                        nc.tensor.transpose(mT[:], mhat[:], identb[:])
                        nc.vector.reduce_max(out1[:, 36 + p:37 + p], mT[:],
                                             axis=AX.X)
                        mn1 = ps_t.tile([L, 1], F32, tag="t")
                        nc.tensor.matmul(mn1[:], mhat[:], ones_colb[:],
                                         start=True, stop=True)
                        nc.vector.tensor_scalar(out1[:, 52 + p:53 + p], mn1[:],
                                                invl2[:, 0:1], None, OP.mult)
                    units.append(per_p)
            return units

        bg_units = mm_units() + mpm_block_units(am2, c1Tb, rw1, W_ATT, out1, 68) \
            + mpm_block_units(am1, c2Tb, rw2, W_ATT, out2, 68)
        bg_iter = iter(bg_units)

        # ---- attentive max loops ----
        # side 1: amx2[i,h] = max_j cosM[i,j]*c2x[j,h]
        # side 2: amx1[j,h] = max_i cosMT[j,i]*c1x[i,h]
        acc1a = sb.tile([L, H], BF16, tag="acc1a")
        nc.gpsimd.memset(acc1a[:], -1e30)
        acc1b = sb.tile([L, H], BF16, tag="acc1b")
        nc.gpsimd.memset(acc1b[:], -1e30)
        acc2a = sb.tile([L, H], BF16, tag="acc2a")
        nc.gpsimd.memset(acc2a[:], -1e30)
        acc2b = sb.tile([L, H], BF16, tag="acc2b")
        nc.gpsimd.memset(acc2b[:], -1e30)

        RING = 8
        ring1 = ring.tile([L, RING, H], BF16, tag="ring1")
        ring2 = ring.tile([L, RING, H], BF16, tag="ring2")
        ring1a = ring.tile([L, RING, H], BF16, tag="ring1a")
        ring2a = ring.tile([L, RING, H], BF16, tag="ring2a")

        # j-indices assigned to the DVE (stt) route vs the ACT route
        dve_js = [j for j in range(L) if (j % 16) < DVE_CNT]
        act_js = [j for j in range(L) if (j % 16) >= DVE_CNT]

        # Pre-issue DMA broadcasts in pairs (2 rows per DMA instruction).
        def issue_bcast(side, k, route="dve"):
            # broadcast rows k, k+1 of the route's j-list into ring slots
            js = (dve_js if route == "dve" else act_js)[k:k + 2]
            if route == "dve":
                rng = ring1 if side == 1 else ring2
            else:
                rng = ring1a if side == 1 else ring2a
            src = dins["c2xd" if side == 1 else "c1xd"]
            eng = nc.sync if side == 1 else nc.gpsimd
            slot = k % RING
            for q, j in enumerate(js):
                eng.dma_start(rng[:, slot + q, :],
                              src[j:j + 1, :].to_broadcast([L, H]))

        def dve_iter(side, k):
            j = dve_js[k]
            rng, acc, cosc = (ring1, acc1a, cosMf) if side == 1 else \
                             (ring2, acc2a, cosMTf)
            nc.vector.scalar_tensor_tensor(
                acc[:], rng[:, k % RING, :], cosc[:, j:j + 1], acc[:],
                OP.mult, OP.max)

        def act_iter(side, k):
            j = act_js[k]
            rng, acc, cosc = (ring1a, acc1b, cosMf) if side == 1 else \
                             (ring2a, acc2b, cosMTf)
            tb = tmpp.tile([L, H], BF16, tag="tbuf")
            nc.scalar.mul(tb[:], rng[:, k % RING, :], cosc[:, j:j + 1])
            nc.vector.tensor_tensor(acc[:], acc[:], tb[:], op=OP.max)

        # prefetch first ring slots for both routes
        for side in (1, 2):
            for k in range(0, min(RING, len(dve_js)), 2):
                issue_bcast(side, k, "dve")
            for k in range(0, min(RING, len(act_js)), 2):
                issue_bcast(side, k, "act")

        nd, na = len(dve_js), len(act_js)
        steps = max(nd, na)
        bg_every = max(1, (2 * steps) // max(len(bg_units), 1) + 1)
        step_no = 0
        for k in range(steps):
            for side in (1, 2):
                if k < nd:
                    dve_iter(side, k)
                    # refill the pair of slots whose readers (k-1, k) have
                    # now both been emitted
                    if (k % 2) == 1 and k + RING - 1 < nd:
                        issue_bcast(side, k + RING - 1, "dve")
                if k < na:
                    act_iter(side, k)
                    if (k % 2) == 1 and k + RING - 1 < na:
                        issue_bcast(side, k + RING - 1, "act")
                step_no += 1
                if step_no % bg_every == 0:
                    u = next(bg_iter, None)
                    if u is not None:
                        u()
        for u in bg_iter:
            u()

        # merge accumulators
        amx2 = acc1a
        nc.vector.tensor_tensor(amx2[:], amx2[:], acc1b[:], op=OP.max)
        amx1 = acc2a
        nc.vector.tensor_tensor(amx1[:], amx1[:], acc2b[:], op=OP.max)

        for u in mpm_block_units(amx2, c1Tb, rw1, W_MATT, out1, 85):
            u()
        for u in mpm_block_units(amx1, c2Tb, rw2, W_MATT, out2, 85):
            u()

        # ---- store (split so only the amx columns gate the tail) ----
        nc.sync.dma_start(dout[0:L, 0:85], out1[:, 0:85])
        nc.sync.dma_start(dout[L:2 * L, 0:85], out2[:, 0:85])
        nc.sync.dma_start(dout[0:L, 85:NF], out1[:, 85:NF])
        nc.sync.dma_start(dout[L:2 * L, 85:NF], out2[:, 85:NF])


_CACHED = None


def _build():
    global _CACHED
    if _CACHED is not None:
        return _CACHED
    nc = bacc.Bacc("TRN2", target_bir_lowering=False, debug=False,
                   enable_asserts=False)
    dins = {}
    for name, shape, dt in [
            ("c1Tb", [L, NCH * L], BF16), ("c2Tb", [L, NCH * L], BF16),
            ("c1b", [L, H], BF16), ("c2b", [L, H], BF16),
            ("c1xb", [L, H], BF16), ("c2xb", [L, H], BF16),
            ("c1xd", [L, H], BF16), ("c2xd", [L, H], BF16),
            ("rhs1", [L, NCH * 34], BF16), ("rhs2", [L, NCH * 34], BF16),
            ("w2tb", [L, NCH * 34], BF16),
            ("rw1", [L, 84], F32), ("rw2", [L, 84], F32),
            ("mone1f", [L, L], F32), ("mone2f", [L, L], F32),
            ("invl1", [L, 1], F32), ("invl2", [L, 1], F32),
            ("negk1", [L, 1], F32), ("negk2", [L, 1], F32),
            ("wc1T4", [L, NCH * PP * L], BF16)]:
        dins[name] = nc.dram_tensor(name, shape, dt, kind="ExternalInput")
    dout = nc.dram_tensor("out", [2 * L, NF], F32, kind="ExternalOutput")
    with tile.TileContext(nc) as tc:
        _trace_kernel(tc, dins, dout[:])
    nc.compile()
    _CACHED = nc
    return nc


def _host_prep(c1raw, m1, c2raw, m2, w_ff, w_fb, w_mp, w_att, w_matt):
    f64 = np.float64
    c1 = (c1raw * m1[:, None]).astype(f64)
    c2 = (c2raw * m2[:, None]).astype(f64)
    len1, len2 = float(m1.sum()), float(m2.sum())
    lp1, lp2 = max(int(len1) - 1, 0), max(int(len2) - 1, 0)

    ones = np.ones((H, 1), f64)
    w2cols = np.concatenate(
        [ones, (w_ff * w_ff).T, ones, (w_fb * w_fb).T,
         ones, (w_att * w_att).T, ones, (w_matt * w_matt).T,
         (w_mp * w_mp).T], 1).astype(f64)          # [H, 84]

    def rw_of(c):
        wn = np.sqrt((c * c) @ w2cols)             # [L, 84]
        return (1.0 / np.maximum(wn, EPS)).astype(np.float32)

    rw1, rw2 = rw_of(c1), rw_of(c2)

    def mpm_rhs(v, w):
        w2 = (w * w).astype(f64)
        rn = 1.0 / max(np.sqrt((v * v).sum()), EPS)
        wn = np.sqrt((w2 * (v * v)[None, :]).sum(1))
        rwn = 1.0 / np.maximum(wn, EPS)
        return np.concatenate(
            [(v * rn)[:, None], (w2 * v[None, :] * rwn[:, None]).T], 1)

    rhs1 = np.concatenate([mpm_rhs(c2[lp2], w_ff), mpm_rhs(c2[0], w_fb)], 1)
    rhs2 = np.concatenate([mpm_rhs(c1[lp1], w_ff), mpm_rhs(c1[0], w_fb)], 1)

    w2tb = np.concatenate(
        [ones, (w_att * w_att).T, ones, (w_matt * w_matt).T], 1)  # [H,34]

    def chunkT(a):  # [L,H] -> [L, NCH, L] transposed-chunk layout, flattened
        return np.ascontiguousarray(
            a.T.reshape(NCH, L, L).transpose(1, 0, 2).reshape(L, NCH * L))

    def chunkR(a):  # [H,k] -> [L, NCH*k]
        k = a.shape[1]
        return np.ascontiguousarray(
            a.reshape(NCH, L, k).transpose(1, 0, 2).reshape(L, NCH * k))

    # wc1T4[hp, c, p, i] = w_mp2[p, c*L+hp] * c1T[c*L+hp, i] * rw1[i, 68+p]
    w_mp2 = (w_mp * w_mp).astype(f64)              # [P, H]
    t = w_mp2.T[:, :, None] * c1.T[:, None, :]     # [H, P, L]
    t = t * rw1[:, MP0:MP0 + PP].T[None, :, :]     # [H, P, i] * rw1[i,p]
    wc1T4 = t.reshape(NCH, L, PP, L).transpose(1, 0, 2, 3).reshape(
        L, NCH * PP * L)

    bc = lambda r: np.ascontiguousarray(
        np.broadcast_to(r[None, :], (L, L)), dtype=np.float32)
    asf = lambda a: np.ascontiguousarray(a, dtype=np.float32)
    asb = lambda a: np.ascontiguousarray(a, dtype=ml_dtypes.bfloat16)
    c1x = c1 + (m1 - 1.0)[:, None] * 1e30
    c2x = c2 + (m2 - 1.0)[:, None] * 1e30
    return dict(
        c1Tb=asb(chunkT(c1)), c2Tb=asb(chunkT(c2)),
        c1b=asb(c1), c2b=asb(c2),
        c1xb=asb(c1x), c2xb=asb(c2x),
        c1xd=asb(c1x), c2xd=asb(c2x),
        rhs1=asb(chunkR(rhs1)), rhs2=asb(chunkR(rhs2)),
        w2tb=asb(chunkR(w2tb)),
        rw1=asf(rw1), rw2=asf(rw2),
        mone1f=bc(1 - m1), mone2f=bc(1 - m2),
        invl1=np.full((L, 1), 1.0 / max(len1, EPS), np.float32),
        invl2=np.full((L, 1), 1.0 / max(len2, EPS), np.float32),
        negk1=np.full((L, 1), -(L - len1), np.float32),
        negk2=np.full((L, 1), -(L - len2), np.float32),
        wc1T4=asb(wc1T4),
    )


def kernel(context_1, mask_1, context_2, mask_2,
           w_ff, w_fb, w_mp, w_att, w_matt, **_unused):
    context_1 = np.asarray(context_1, dtype=np.float32)
    context_2 = np.asarray(context_2, dtype=np.float32)
    mask_1 = np.asarray(mask_1, dtype=np.float32)
    mask_2 = np.asarray(mask_2, dtype=np.float32)
    w_ff, w_fb = np.asarray(w_ff, np.float32), np.asarray(w_fb, np.float32)
    w_mp = np.asarray(w_mp, np.float32)
    w_att, w_matt = np.asarray(w_att, np.float32), np.asarray(w_matt, np.float32)
    assert context_1.shape == (B, L, H), context_1.shape

    nc = _build()
    in_maps = [
        _host_prep(context_1[b], mask_1[b], context_2[b], mask_2[b],
                   w_ff, w_fb, w_mp, w_att, w_matt)
        for b in range(B)
    ]
    res = run_bass_kernel_spmd(nc, in_maps, core_ids=list(range(B)))
    global LAST_RESULTS
    LAST_RESULTS = res
    return np.stack([res.results[b]["out"] for b in range(B)]).astype(np.float32)


LAST_RESULTS = None


if __name__ == "__main__":
    rng = np.random.default_rng(0)
    ins = dict(
        context_1=rng.standard_normal((B, L, H), dtype=np.float32),
        context_2=rng.standard_normal((B, L, H), dtype=np.float32),
        mask_1=(np.arange(L)[None, :] < rng.integers(64, 129, B)[:, None]
                ).astype(np.float32),
        mask_2=(np.arange(L)[None, :] < rng.integers(64, 129, B)[:, None]
                ).astype(np.float32),
        w_ff=rng.standard_normal((PP, H), dtype=np.float32) * 0.05,
        w_fb=rng.standard_normal((PP, H), dtype=np.float32) * 0.05,
        w_mp=rng.standard_normal((PP, H), dtype=np.float32) * 0.05,
        w_att=rng.standard_normal((PP, H), dtype=np.float32) * 0.05,
        w_matt=rng.standard_normal((PP, H), dtype=np.float32) * 0.05,
    )
    out = kernel(**ins)
    print("out", out.shape, out.dtype, np.abs(out).max())
